# revision 9
# baseline (speedup 1.0000x reference)
"""Trainium2 Bass kernel for multi-head cross-attention.

Problem: q [4, 2048, 512], kv [4, 4096, 128], 8 heads x 64 dim,
out = softmax((q Wq^T)(kv Wk^T)^T / 8) (kv Wv^T) Wo^T + b_o.

Sharding: pure data parallel over 8 NeuronCores; core c handles batch c//2,
query rows (c%2)*1024:(c%2+1)*1024. No collectives.

Per-core dataflow (all layouts feature-major so the PE contracts on partitions):
  - PE-transpose q, kv and the weights into SBUF (bf16).
  - Projections QT=[e,i], KT=[e,j], V=[j,e] via bf16 matmuls.
  - Per head: S^T[j,i] = KT_h^T-free matmul; ACT exp (scale=1/8 folded in,
    no max-subtract needed: logits ~ N(0,1), fp32 exp is safe);
    AV uses V augmented with a ones column so the softmax denominator
    falls out of the same matmul (row 64 of the [65, i] accumulator).
  - Division by the denominator happens after AV (it commutes), via DVE
    reciprocal + SBUF->SBUF DMA partition-broadcast + tensor_mul.
  - out_linear accumulates 8 K=64 matmuls; bias added on DVE.
"""

import sys

import numpy as np

for _p in ("/opt/trn_rl_repo",):
    if _p not in sys.path:
        sys.path.insert(0, _p)

import concourse.bass as bass
import concourse.tile as tile
from concourse import mybir
from concourse.bass_utils import run_bass_kernel_spmd

F32 = mybir.dt.float32
BF16 = mybir.dt.bfloat16

B = 4
NQ_FULL = 2048
NQ = 1024          # queries per core
NK = 4096
C = 512            # q feature dim
KVC = 128          # kv feature dim
H = 8
D = 64
INNER = H * D      # 512
SCALE = D ** -0.5  # 0.125
N_CORES = 8


def _emit(tc, nc, q, kv, w_q, w_k, w_v, w_o, b_o, out):
    Exp = mybir.ActivationFunctionType.Exp

    with (
        tc.tile_pool(name="const", bufs=1) as const,
        tc.tile_pool(name="stage", bufs=3) as stage,
        tc.tile_pool(name="big", bufs=1) as big,
        tc.tile_pool(name="epool", bufs=4) as epool,
        tc.tile_pool(name="hdr", bufs=2) as hdr,
        tc.tile_pool(name="outp", bufs=2) as outp,
        tc.tile_pool(name="dscr", bufs=2, space="DRAM") as dscr,
        tc.tile_pool(name="ps", bufs=2, space="PSUM") as ps,
        tc.tile_pool(name="avps", bufs=2, space="PSUM") as avps,
    ):
        bias_bc = const.tile([128, C], F32)
        nc.gpsimd.dma_start(out=bias_bc, in_=b_o.to_broadcast([128, C]))

        # Transposed-weight SBUF tiles
        wqT = big.tile([128, 4, C], BF16)      # [c_in, cb, e]
        wkT = big.tile([128, C], BF16)         # [c, e]
        wvT = big.tile([128, C], BF16)         # [c, e]
        woT128 = big.tile([128, 4, C], BF16)   # [e_in, eb, o]
        woT = big.tile([64, H, C], BF16)       # [e_in, h, o] (base-0 per head)
        qTin = big.tile([128, 4, NQ], BF16)    # [c_in, cb, i]
        kvT = big.tile([128, NK], BF16)        # [c, j]

        # ---- loads: f32 -> DVE cast to bf16 -> hardware DMA transpose ----
        def load_cast(src_ap, shape, tagf, tagb):
            t_f = stage.tile(shape, F32, tag=tagf, name="t_f")
            nc.sync.dma_start(out=t_f, in_=src_ap)
            t_b = stage.tile(shape, BF16, tag=tagb, name="t_b")
            nc.vector.tensor_copy(t_b, t_f)
            return t_b

        for eb in range(4):
            wq_b = load_cast(w_q[eb * 128:(eb + 1) * 128, :], [128, C],
                             "stage", "stageb")
            for cb in range(4):
                nc.sync.dma_start_transpose(
                    wqT[:, cb, eb * 128:(eb + 1) * 128],
                    wq_b[:, cb * 128:(cb + 1) * 128])
        for eb in range(4):
            wk_b = load_cast(w_k[eb * 128:(eb + 1) * 128, :], [128, KVC],
                             "stage", "stageb")
            nc.sync.dma_start_transpose(wkT[:, eb * 128:(eb + 1) * 128], wk_b)
        for eb in range(4):
            wv_b = load_cast(w_v[eb * 128:(eb + 1) * 128, :], [128, KVC],
                             "stage", "stageb")
            nc.sync.dma_start_transpose(wvT[:, eb * 128:(eb + 1) * 128], wv_b)
        for ob in range(4):
            wo_b = load_cast(w_o[ob * 128:(ob + 1) * 128, :], [128, C],
                             "stage", "stageb")
            for eb in range(4):
                nc.sync.dma_start_transpose(
                    woT128[:, eb, ob * 128:(ob + 1) * 128],
                    wo_b[:, eb * 128:(eb + 1) * 128])
        # relocate head halves to base partition 0 (plain SBUF->SBUF DMA)
        for t in range(4):
            nc.sync.dma_start(out=woT[:, 2 * t, :], in_=woT128[0:64, t, :])
            nc.sync.dma_start(out=woT[:, 2 * t + 1, :], in_=woT128[64:128, t, :])

        for ib in range(8):
            q_b = load_cast(q[ib * 128:(ib + 1) * 128, :], [128, C],
                            "stage", "stageb")
            for cb in range(4):
                nc.sync.dma_start_transpose(
                    qTin[:, cb, ib * 128:(ib + 1) * 128],
                    q_b[:, cb * 128:(cb + 1) * 128])
        for ch in range(4):
            kv_b = load_cast(
                kv[ch * 1024:(ch + 1) * 1024, :].rearrange(
                    "(a p) d -> p a d", p=128),
                [128, 8, KVC], "kvst", "kvstb")
            for a in range(8):
                jb = ch * 8 + a
                nc.sync.dma_start_transpose(kvT[:, jb * 128:(jb + 1) * 128],
                                            kv_b[:, a, :])

        # ---- projections ----
        KT = big.tile([128, 4, NK], BF16)    # [e_in, et, j]
        QT = big.tile([128, 4, NQ], BF16)    # [e_in, et, i]
        vaug = big.tile([128, 32, H, 65], BF16)  # [j_in, jc, h, d|ones]
        nc.vector.memset(vaug[:, :, :, 64:65], 1.0)

        for et in range(4):
            for jc in range(8):
                kt_ps = ps.tile([128, 512], F32, tag="sx", name="kt_ps")
                nc.tensor.matmul(kt_ps,
                                 lhsT=wkT[:, et * 128:(et + 1) * 128],
                                 rhs=kvT[:, jc * 512:(jc + 1) * 512],
                                 start=True, stop=True)
                nc.vector.tensor_copy(KT[:, et, jc * 512:(jc + 1) * 512], kt_ps)

        for et in range(4):
            for ih in range(2):
                qt_ps = ps.tile([128, 512], F32, tag="sx", name="qt_ps")
                for cb in range(4):
                    nc.tensor.matmul(qt_ps,
                                     lhsT=wqT[:, cb, et * 128:(et + 1) * 128],
                                     rhs=qTin[:, cb, ih * 512:(ih + 1) * 512],
                                     start=(cb == 0), stop=(cb == 3))
                nc.vector.tensor_copy(QT[:, et, ih * 512:(ih + 1) * 512], qt_ps)

        for jc in range(32):
            v_ps = ps.tile([128, 512], F32, tag="sx", name="v_ps")
            nc.tensor.matmul(v_ps,
                             lhsT=kvT[:, jc * 128:(jc + 1) * 128],
                             rhs=wvT,
                             start=True, stop=True)
            nc.vector.tensor_copy(vaug[:, jc, :, 0:64],
                                  v_ps.rearrange("p (h d) -> p h d", h=H))

        # ---- attention head loop ----
        aoT = big.tile([64, H, NQ], BF16)    # attn-out^T: [d, h, i]
        for h in range(H):
            et, row = h // 2, (h % 2) * 64
            av_ps = avps.tile([65, NQ], F32, name="av_ps")
            for jc in range(32):
                st_ps = ps.tile([128, NQ], F32, tag="sx", name="st_ps")
                for ih in range(2):
                    nc.tensor.matmul(
                        st_ps[:, ih * 512:(ih + 1) * 512],
                        lhsT=KT[row:row + 64, et, jc * 128:(jc + 1) * 128],
                        rhs=QT[row:row + 64, et, ih * 512:(ih + 1) * 512],
                        start=True, stop=True)
                expS = epool.tile([128, NQ], BF16, name="expS")
                nc.scalar.activation(out=expS, in_=st_ps, func=Exp, scale=SCALE)
                for ih in range(2):
                    nc.tensor.matmul(
                        av_ps[:, ih * 512:(ih + 1) * 512],
                        lhsT=vaug[:, jc, h, :],
                        rhs=expS[:, ih * 512:(ih + 1) * 512],
                        start=(jc == 0), stop=(jc == 31))
            # softmax denominator: row 64 of av_ps
            rec = hdr.tile([65, NQ], F32, tag="rec", name="rec")
            nc.vector.reciprocal(rec[64:65, :], av_ps[64:65, :])
            rdram = dscr.tile([1, NQ], F32, name="rdram")
            nc.sync.dma_start(out=rdram, in_=rec[64:65, :])
            rbc = hdr.tile([64, NQ], F32, tag="rbc", name="rbc")
            nc.gpsimd.dma_start(out=rbc, in_=rdram.to_broadcast([64, NQ]))
            nc.vector.tensor_mul(aoT[:, h, :], av_ps[0:64, :], rbc)

        # ---- out linear ----
        for ic in range(8):
            ol_ps = ps.tile([128, 512], F32, tag="sx", name="ol_ps")
            for h in range(H):
                nc.tensor.matmul(ol_ps,
                                 lhsT=aoT[:, h, ic * 128:(ic + 1) * 128],
                                 rhs=woT[:, h, :],
                                 start=(h == 0), stop=(h == H - 1))
            o_sb = outp.tile([128, C], F32, name="o_sb")
            nc.vector.tensor_add(o_sb, ol_ps, bias_bc)
            nc.sync.dma_start(out=out[ic * 128:(ic + 1) * 128, :], in_=o_sb)


def build_program():
    from concourse import bacc
    nc = bacc.Bacc("TRN2", target_bir_lowering=False, debug=False)
    q = nc.dram_tensor("q", [NQ, C], F32, kind="ExternalInput").ap()
    kv = nc.dram_tensor("kv", [NK, KVC], F32, kind="ExternalInput").ap()
    w_q = nc.dram_tensor("w_q", [INNER, C], F32, kind="ExternalInput").ap()
    w_k = nc.dram_tensor("w_k", [INNER, KVC], F32, kind="ExternalInput").ap()
    w_v = nc.dram_tensor("w_v", [INNER, KVC], F32, kind="ExternalInput").ap()
    w_o = nc.dram_tensor("w_o", [C, INNER], F32, kind="ExternalInput").ap()
    b_o = nc.dram_tensor("b_o", [1, C], F32, kind="ExternalInput").ap()
    out = nc.dram_tensor("out", [NQ, C], F32, kind="ExternalOutput").ap()
    with tile.TileContext(nc) as tc:
        _emit(tc, nc, q, kv, w_q, w_k, w_v, w_o, b_o, out)
    nc.compile()
    return nc


def make_in_maps(q, kv, w_q, w_k, w_v, w_o, b_o):
    q = np.ascontiguousarray(q, dtype=np.float32)
    kv = np.ascontiguousarray(kv, dtype=np.float32)
    w_q = np.ascontiguousarray(w_q, dtype=np.float32)
    w_k = np.ascontiguousarray(w_k, dtype=np.float32)
    w_v = np.ascontiguousarray(w_v, dtype=np.float32)
    w_o = np.ascontiguousarray(w_o, dtype=np.float32)
    b_o = np.ascontiguousarray(b_o, dtype=np.float32).reshape(1, C)
    in_maps = []
    for core in range(N_CORES):
        b, half = core // 2, core % 2
        in_maps.append({
            "q": np.ascontiguousarray(q[b, half * NQ:(half + 1) * NQ]),
            "kv": kv[b],
            "w_q": w_q, "w_k": w_k, "w_v": w_v, "w_o": w_o, "b_o": b_o,
        })
    return in_maps


def assemble(results):
    out = np.zeros((B, NQ_FULL, C), np.float32)
    for core in range(N_CORES):
        b, half = core // 2, core % 2
        out[b, half * NQ:(half + 1) * NQ] = results[core]["out"]
    return out


def run(inputs, trace=False, **kwargs):
    nc = build_program()
    in_maps = make_in_maps(**inputs)
    res = run_bass_kernel_spmd(nc, in_maps, core_ids=list(range(N_CORES)),
                               trace=trace, **kwargs)
    return assemble(res.results), res


def kernel(q, kv, w_q, w_k, w_v, w_o, b_o):
    out, _ = run(dict(q=q, kv=kv, w_q=w_q, w_k=w_k, w_v=w_v, w_o=w_o, b_o=b_o))
    return out


# revision 13
# speedup vs baseline: 1.5135x; 1.5135x over previous
"""Trainium2 Bass kernel for multi-head cross-attention.

Problem: q [4, 2048, 512], kv [4, 4096, 128], 8 heads x 64 dim,
out = softmax((q Wq^T)(kv Wk^T)^T / 8) (kv Wv^T) Wo^T + b_o.

Sharding: pure data parallel over 8 NeuronCores; core c handles batch c//2,
query rows (c%2)*1024:(c%2+1)*1024. No collectives.

Per-core dataflow (all layouts feature-major so the PE contracts on partitions):
  - PE-transpose q, kv and the weights into SBUF (bf16).
  - Projections QT=[e,i], KT=[e,j], V=[j,e] via bf16 matmuls.
  - Per head: S^T[j,i] = KT_h^T-free matmul; ACT exp (scale=1/8 folded in,
    no max-subtract needed: logits ~ N(0,1), fp32 exp is safe);
    AV uses V augmented with a ones column so the softmax denominator
    falls out of the same matmul (row 64 of the [65, i] accumulator).
  - Division by the denominator happens after AV (it commutes), via DVE
    reciprocal + SBUF->SBUF DMA partition-broadcast + tensor_mul.
  - out_linear accumulates 8 K=64 matmuls; bias added on DVE.
"""

import sys

import numpy as np

for _p in ("/opt/trn_rl_repo",):
    if _p not in sys.path:
        sys.path.insert(0, _p)

import concourse.bass as bass
import concourse.tile as tile
from concourse import mybir
from concourse.bass_utils import run_bass_kernel_spmd

F32 = mybir.dt.float32
BF16 = mybir.dt.bfloat16

B = 4
NQ_FULL = 2048
NQ = 1024          # queries per core
NK = 4096
C = 512            # q feature dim
KVC = 128          # kv feature dim
H = 8
D = 64
INNER = H * D      # 512
SCALE = D ** -0.5  # 0.125
N_CORES = 8
DEBUG_DUMP = False


def _emit(tc, nc, q, kv, w_q, w_k, w_v, w_o, b_o, out):
    Exp = mybir.ActivationFunctionType.Exp

    with (
        tc.tile_pool(name="const", bufs=1) as const,
        tc.tile_pool(name="stage", bufs=3) as stage,
        tc.tile_pool(name="big", bufs=1) as big,
        tc.tile_pool(name="epool", bufs=4) as epool,
        tc.tile_pool(name="hdr", bufs=2) as hdr,
        tc.tile_pool(name="outp", bufs=2) as outp,
        tc.tile_pool(name="dscr", bufs=2, space="DRAM") as dscr,
        tc.tile_pool(name="ps", bufs=2, space="PSUM") as ps,
        tc.tile_pool(name="avps", bufs=2, space="PSUM") as avps,
    ):
        # Layouts (feature-major so the PE contracts over partitions):
        wqT = big.tile([128, 4, C], BF16)      # [c_in, cb, e]
        wkT = big.tile([128, C], BF16)         # [c, e]
        wvT = big.tile([128, C], BF16)         # [c, e]
        qTin = big.tile([128, 4, NQ], BF16)    # [c_in, cb, i]
        kvT = big.tile([128, NK], BF16)        # [c, j]
        KT = big.tile([128, 4, NK], BF16)      # [e_in, et, j]
        QT = big.tile([128, 4, NQ], BF16)      # [e_in, et, i]
        vaug = big.tile([128, 32, H, 65], BF16)  # [j_in, jc, h, d|ones]
        aoT = big.tile([64, H, NQ], BF16)      # attn-out^T per head, base 0

        # ---- kv: cast-load (SWDGE) + one xbar transpose per 1024 rows ----
        for ch in range(4):
            kv_b = stage.tile([128, 8, KVC], BF16, tag="kvst", bufs=2,
                              name="kv_b")
            nc.gpsimd.dma_start(
                out=kv_b,
                in_=kv[ch * 1024:(ch + 1) * 1024, :].rearrange(
                    "(a p) d -> p a d", p=128))
            nc.sync.dma_start_transpose(
                kvT[:, ch * 1024:(ch + 1) * 1024].rearrange(
                    "p (a j) -> p a j", a=8),
                kv_b)

        # ---- w_k, w_v: one cast-load + one transpose each ----
        wk_b = stage.tile([128, 4, KVC], BF16, tag="wkv", name="wk_b")
        nc.gpsimd.dma_start(out=wk_b,
                            in_=w_k.rearrange("(eb p) c -> p eb c", p=128))
        nc.sync.dma_start_transpose(
            wkT.rearrange("p (eb e) -> p eb e", eb=4), wk_b)
        wv_b = stage.tile([128, 4, KVC], BF16, tag="wkv", name="wv_b")
        nc.gpsimd.dma_start(out=wv_b,
                            in_=w_v.rearrange("(eb p) c -> p eb c", p=128))
        nc.sync.dma_start_transpose(
            wvT.rearrange("p (eb e) -> p eb e", eb=4), wv_b)

        # ---- KT projection ----
        for et in range(4):
            for jc in range(8):
                kt_ps = ps.tile([128, 512], F32, tag="sx", name="kt_ps")
                nc.tensor.matmul(kt_ps,
                                 lhsT=wkT[:, et * 128:(et + 1) * 128],
                                 rhs=kvT[:, jc * 512:(jc + 1) * 512],
                                 start=True, stop=True)
                nc.vector.tensor_copy(KT[:, et, jc * 512:(jc + 1) * 512], kt_ps)

        # ---- q + w_q: cast-loads + transposes ----
        for ib in range(8):
            q_b = stage.tile([128, C], BF16, tag="stage", name="q_b")
            nc.gpsimd.dma_start(out=q_b, in_=q[ib * 128:(ib + 1) * 128, :])
            nc.sync.dma_start_transpose(qTin[:, :, ib * 128:(ib + 1) * 128],
                                        q_b)
        for eb in range(4):
            wq_b = stage.tile([128, C], BF16, tag="stage", name="wq_b")
            nc.gpsimd.dma_start(out=wq_b, in_=w_q[eb * 128:(eb + 1) * 128, :])
            nc.sync.dma_start_transpose(wqT[:, :, eb * 128:(eb + 1) * 128],
                                        wq_b)

        # ---- QT projection ----
        for et in range(4):
            for ih in range(2):
                qt_ps = ps.tile([128, 512], F32, tag="sx", name="qt_ps")
                for cb in range(4):
                    nc.tensor.matmul(qt_ps,
                                     lhsT=wqT[:, cb, et * 128:(et + 1) * 128],
                                     rhs=qTin[:, cb, ih * 512:(ih + 1) * 512],
                                     start=(cb == 0), stop=(cb == 3))
                nc.vector.tensor_copy(QT[:, et, ih * 512:(ih + 1) * 512], qt_ps)

        # ---- V projection (interleaves with early head-loop work) ----
        nc.vector.memset(vaug[:, :, :, 64:65], 1.0)
        for jc in range(32):
            v_ps = ps.tile([128, 512], F32, tag="sx", name="v_ps")
            nc.tensor.matmul(v_ps,
                             lhsT=kvT[:, jc * 128:(jc + 1) * 128],
                             rhs=wvT,
                             start=True, stop=True)
            nc.vector.tensor_copy(vaug[:, jc, :, 0:64],
                                  v_ps.rearrange("p (h d) -> p h d", h=H))

        # ---- attention: heads in pairs to keep the PE dense ----
        def head_tail(h, av_ps):
            # softmax denominator lives in row 64 of av_ps. PSUM fp32 is not
            # bit-safe for the custom bitwise reciprocal -> stage via SBUF.
            srow = hdr.tile([65, NQ], F32, tag="srow", name="srow")
            nc.vector.tensor_copy(srow[64:65, :], av_ps[64:65, :])
            rdram = dscr.tile([1, NQ], F32, name="rdram")
            nc.gpsimd.dma_start(out=rdram, in_=srow[64:65, :])
            rbc = hdr.tile([64, NQ], F32, tag="rbc", name="rbc")
            nc.gpsimd.dma_start(out=rbc, in_=rdram.to_broadcast([64, NQ]))
            rbcr = hdr.tile([64, NQ], F32, tag="rbcr", name="rbcr")
            nc.vector.reciprocal_approx_fast(rbcr, rbc)
            nc.vector.tensor_mul(aoT[:, h, :], av_ps[0:64, :], rbcr)

        for hp in range(4):
            et = hp
            avs = [avps.tile([65, NQ], F32, name="av_ps") for _ in range(2)]
            for jc in range(32):
                exps = []
                for k in range(2):
                    row = k * 64
                    st_ps = ps.tile([128, NQ], F32, tag="sx", name="st_ps")
                    for ih in range(2):
                        nc.tensor.matmul(
                            st_ps[:, ih * 512:(ih + 1) * 512],
                            lhsT=KT[row:row + 64, et, jc * 128:(jc + 1) * 128],
                            rhs=QT[row:row + 64, et, ih * 512:(ih + 1) * 512],
                            start=True, stop=True)
                    expS = epool.tile([128, NQ], BF16, name="expS")
                    nc.scalar.activation(out=expS, in_=st_ps, func=Exp,
                                         scale=SCALE)
                    exps.append(expS)
                for k in range(2):
                    for ih in range(2):
                        nc.tensor.matmul(
                            avs[k][:, ih * 512:(ih + 1) * 512],
                            lhsT=vaug[:, jc, 2 * hp + k, :],
                            rhs=exps[k][:, ih * 512:(ih + 1) * 512],
                            start=(jc == 0), stop=(jc == 31))
            for k in range(2):
                head_tail(2 * hp + k, avs[k])

        # ---- w_o + bias (needed only now) ----
        woT128 = big.tile([128, 4, C], BF16)   # [e_in, eb, o]
        woT = big.tile([64, H, C], BF16)       # [e_in, h, o] (base-0 per head)
        for ob in range(4):
            wo_b = stage.tile([128, C], BF16, tag="stage", name="wo_b")
            nc.gpsimd.dma_start(out=wo_b, in_=w_o[ob * 128:(ob + 1) * 128, :])
            nc.sync.dma_start_transpose(woT128[:, :, ob * 128:(ob + 1) * 128],
                                        wo_b)
        # relocate odd-head halves to base partition 0 via DRAM round-trip
        wo_scr = dscr.tile([64, H, C], BF16, name="wo_scr")
        for t in range(4):
            nc.gpsimd.dma_start(out=wo_scr[:, 2 * t, :], in_=woT128[0:64, t, :])
            nc.gpsimd.dma_start(out=wo_scr[:, 2 * t + 1, :],
                                in_=woT128[64:128, t, :])
        nc.gpsimd.dma_start(out=woT, in_=wo_scr)
        bias_bc = const.tile([128, C], F32)
        nc.gpsimd.dma_start(out=bias_bc, in_=b_o.to_broadcast([128, C]))

        if DEBUG_DUMP:
            for nm, t in [("kvT", kvT), ("wkT", wkT), ("wvT", wvT),
                          ("qTin", qTin), ("wqT", wqT), ("KT", KT),
                          ("QT", QT), ("vaug", vaug), ("aoT", aoT),
                          ("woT", woT)]:
                dd = nc.dram_tensor("d_" + nm, list(t.shape), t.dtype,
                                    kind="ExternalOutput").ap()
                nc.gpsimd.dma_start(out=dd, in_=t)

        # ---- out linear ----
        for ic in range(8):
            ol_ps = ps.tile([128, 512], F32, tag="sx", name="ol_ps")
            for h in range(H):
                nc.tensor.matmul(ol_ps,
                                 lhsT=aoT[:, h, ic * 128:(ic + 1) * 128],
                                 rhs=woT[:, h, :],
                                 start=(h == 0), stop=(h == H - 1))
            o_sb = outp.tile([128, C], F32, name="o_sb")
            nc.vector.tensor_add(o_sb, ol_ps, bias_bc)
            nc.gpsimd.dma_start(out=out[ic * 128:(ic + 1) * 128, :], in_=o_sb)


def build_program():
    from concourse import bacc
    nc = bacc.Bacc("TRN2", target_bir_lowering=False, debug=False)
    q = nc.dram_tensor("q", [NQ, C], F32, kind="ExternalInput").ap()
    kv = nc.dram_tensor("kv", [NK, KVC], F32, kind="ExternalInput").ap()
    w_q = nc.dram_tensor("w_q", [INNER, C], F32, kind="ExternalInput").ap()
    w_k = nc.dram_tensor("w_k", [INNER, KVC], F32, kind="ExternalInput").ap()
    w_v = nc.dram_tensor("w_v", [INNER, KVC], F32, kind="ExternalInput").ap()
    w_o = nc.dram_tensor("w_o", [C, INNER], F32, kind="ExternalInput").ap()
    b_o = nc.dram_tensor("b_o", [1, C], F32, kind="ExternalInput").ap()
    out = nc.dram_tensor("out", [NQ, C], F32, kind="ExternalOutput").ap()
    with tile.TileContext(nc) as tc:
        _emit(tc, nc, q, kv, w_q, w_k, w_v, w_o, b_o, out)
    nc.compile()
    return nc


def make_in_maps(q, kv, w_q, w_k, w_v, w_o, b_o):
    q = np.ascontiguousarray(q, dtype=np.float32)
    kv = np.ascontiguousarray(kv, dtype=np.float32)
    w_q = np.ascontiguousarray(w_q, dtype=np.float32)
    w_k = np.ascontiguousarray(w_k, dtype=np.float32)
    w_v = np.ascontiguousarray(w_v, dtype=np.float32)
    w_o = np.ascontiguousarray(w_o, dtype=np.float32)
    b_o = np.ascontiguousarray(b_o, dtype=np.float32).reshape(1, C)
    in_maps = []
    for core in range(N_CORES):
        b, half = core // 2, core % 2
        in_maps.append({
            "q": np.ascontiguousarray(q[b, half * NQ:(half + 1) * NQ]),
            "kv": kv[b],
            "w_q": w_q, "w_k": w_k, "w_v": w_v, "w_o": w_o, "b_o": b_o,
        })
    return in_maps


def assemble(results):
    out = np.zeros((B, NQ_FULL, C), np.float32)
    for core in range(N_CORES):
        b, half = core // 2, core % 2
        out[b, half * NQ:(half + 1) * NQ] = results[core]["out"]
    return out


def run(inputs, trace=False, **kwargs):
    nc = build_program()
    in_maps = make_in_maps(**inputs)
    res = run_bass_kernel_spmd(nc, in_maps, core_ids=list(range(N_CORES)),
                               trace=trace, **kwargs)
    return assemble(res.results), res


def kernel(q, kv, w_q, w_k, w_v, w_o, b_o):
    out, _ = run(dict(q=q, kv=kv, w_q=w_q, w_k=w_k, w_v=w_v, w_o=w_o, b_o=b_o))
    return out
